# revision 6
# baseline (speedup 1.0000x reference)
"""Haar DWT (2x2 block transform) for Trainium2, data-parallel over 8 NeuronCores.

Full input x: (16, 64, 256, 256) fp32 -> output (16, 256, 128, 128) fp32 where
out[b, 4c+k] = subband k of channel c, k in [cA, cH, cV, cD].

Sharding: batch dim 16 -> 2 per core. Per core the (2, 64) batch/channel dims
flatten to exactly 128 images = the SBUF partition dim; each partition owns one
256x256 image laid out contiguously in its free dim.

Per-core pipeline (per 32-row tile of every image):
  1. DMA in  (128, 8192) fp32               [nc.sync / HWDGE, 4 MiB contiguous]
  2. ScalarE: x *= 0.5 in place             [folds the Haar 1/2 scale]
  3. VectorE: u = top+bot, v = bot-top      [vertical butterfly, unit stride]
  4. VectorE: even+odd -> [cA|cH], odd-even -> [cV|cD]   [horizontal butterfly]
  5. DMA out (128, 4x2048) to the 4 subband regions in one strided store
"""

import numpy as np

B, C, H, W = 16, 64, 256, 256
N_CORES = 8
B_PER = B // N_CORES  # 2
IMGS = B_PER * C  # 128 images/core = SBUF partitions
IMG_PIX = H * W  # 65536 elements per image
ROWS_PER_TILE = 32
K = ROWS_PER_TILE * W  # 8192 free elems / partition / tile
NTILES = H // ROWS_PER_TILE  # 8
SUB = (H // 2) * (W // 2)  # 16384 elements per subband
OUT_K = K // 4  # 2048 output elems per subband per tile

_CACHE: dict = {}


def build_nc():
    import concourse.bacc as bacc
    import concourse.mybir as mybir
    from concourse.tile import TileContext

    fp32 = mybir.dt.float32
    # Bacc (not plain Bass): its generate_event_semaphores pass splits
    # multi-sem waits, which the TRN2 static-DMA encoding can't hold.
    nc = bacc.Bacc(target_bir_lowering=False, debug=False)
    x = nc.dram_tensor("x", [IMGS, IMG_PIX], fp32, kind="ExternalInput")
    y = nc.dram_tensor("y", [IMGS, 4 * SUB], fp32, kind="ExternalOutput")
    # y viewed per subband: (128, 4, 16384)
    y_sub = y[:].rearrange("p (k s) -> p k s", k=4)

    with TileContext(nc) as tc:
        with (
            tc.tile_pool(name="xt", bufs=2) as pool_x,
            tc.tile_pool(name="uv", bufs=2) as pool_uv,
            tc.tile_pool(name="res", bufs=2) as pool_res,
        ):
            for t in range(NTILES):
                xt = pool_x.tile([IMGS, K], fp32)
                nc.gpsimd.dma_start(out=xt[:], in_=x[:, t * K : (t + 1) * K])

                # vertical butterfly: row pairs (2i, 2i+1), unit-stride operands
                xv = xt[:].rearrange("p (i w) -> p i w", w=2 * W)  # (128, 16, 512)
                top = xv[:, :, 0:W]
                bot = xv[:, :, W : 2 * W]
                uv = pool_uv.tile([IMGS, K], fp32)
                u = uv[:, 0 : K // 2].rearrange("p (i w) -> p i w", w=W)
                v = uv[:, K // 2 : K].rearrange("p (i w) -> p i w", w=W)
                nc.vector.tensor_add(out=u, in0=top, in1=bot)  # a+c, b+d
                nc.vector.tensor_sub(out=v, in0=bot, in1=top)  # c-a, d-b
                # fold the Haar 1/2 on ScalarE, keeping DMAs single-dependency:
                # xt is only ever read by DVE, res only written by DVE.
                nc.scalar.mul(uv[:], uv[:], 0.5)

                # horizontal butterfly: column pairs; same op serves both halves
                uvp = uv[:].rearrange("p (n u) -> p n u", u=2)
                even = uvp[:, :, 0]
                odd = uvp[:, :, 1]
                res = pool_res.tile([IMGS, K], fp32)
                nc.vector.tensor_add(out=res[:, 0 : K // 2], in0=even, in1=odd)  # [cA|cH]
                nc.vector.tensor_sub(out=res[:, K // 2 : K], in0=odd, in1=even)  # [cV|cD]

                # res = [cA|cH|cV|cD] x 2048; one strided store to all 4 subbands
                dst = y_sub[:, :, t * OUT_K : (t + 1) * OUT_K]  # (128, 4, 2048)
                src = res[:].rearrange("p (k o) -> p k o", k=4)
                nc.gpsimd.dma_start(out=dst, in_=src)
    # run Bacc's pass pipeline (regalloc, DCE, event-semaphore wait splitting)
    nc.compile()
    return nc


def _get_nc():
    if "nc" not in _CACHE:
        _CACHE["nc"] = build_nc()
    return _CACHE["nc"]


def kernel(x: np.ndarray) -> np.ndarray:
    from concourse.bass_utils import run_bass_kernel_spmd

    x = np.ascontiguousarray(np.asarray(x), dtype=np.float32)
    assert x.shape == (B, C, H, W), x.shape

    nc = _get_nc()
    in_maps = [
        {"x": x[c * B_PER : (c + 1) * B_PER].reshape(IMGS, IMG_PIX)}
        for c in range(N_CORES)
    ]
    results = run_bass_kernel_spmd(nc, in_maps, core_ids=list(range(N_CORES))).results
    out = np.concatenate(
        [r["y"].reshape(B_PER, C * 4, H // 2, W // 2) for r in results], axis=0
    )
    return out


# revision 7
# speedup vs baseline: 1.0338x; 1.0338x over previous
"""Haar DWT (2x2 block transform) for Trainium2, data-parallel over 8 NeuronCores.

Full input x: (16, 64, 256, 256) fp32 -> output (16, 256, 128, 128) fp32 where
out[b, 4c+k] = subband k of channel c, k in [cA, cH, cV, cD].

Sharding: batch dim 16 -> 2 per core. Per core the (2, 64) batch/channel dims
flatten to exactly 128 images = the SBUF partition dim; each partition owns one
256x256 image laid out contiguously in its free dim.

Per-core pipeline (per 32-row tile of every image):
  1. DMA in  (128, 8192) fp32               [nc.sync / HWDGE, 4 MiB contiguous]
  2. ScalarE: x *= 0.5 in place             [folds the Haar 1/2 scale]
  3. VectorE: u = top+bot, v = bot-top      [vertical butterfly, unit stride]
  4. VectorE: even+odd -> [cA|cH], odd-even -> [cV|cD]   [horizontal butterfly]
  5. DMA out (128, 4x2048) to the 4 subband regions in one strided store
"""

import numpy as np

B, C, H, W = 16, 64, 256, 256
N_CORES = 8
B_PER = B // N_CORES  # 2
IMGS = B_PER * C  # 128 images/core = SBUF partitions
IMG_PIX = H * W  # 65536 elements per image
ROWS_PER_TILE = 32
K = ROWS_PER_TILE * W  # 8192 free elems / partition / tile
NTILES = H // ROWS_PER_TILE  # 8
SUB = (H // 2) * (W // 2)  # 16384 elements per subband
OUT_K = K // 4  # 2048 output elems per subband per tile

_CACHE: dict = {}


def build_nc():
    import concourse.bacc as bacc
    import concourse.mybir as mybir
    from concourse.tile import TileContext

    fp32 = mybir.dt.float32
    # Bacc (not plain Bass): its generate_event_semaphores pass splits
    # multi-sem waits, which the TRN2 static-DMA encoding can't hold.
    nc = bacc.Bacc(target_bir_lowering=False, debug=False)
    x = nc.dram_tensor("x", [IMGS, IMG_PIX], fp32, kind="ExternalInput")
    y = nc.dram_tensor("y", [IMGS, 4 * SUB], fp32, kind="ExternalOutput")
    # y viewed per subband: (128, 4, 16384)
    y_sub = y[:].rearrange("p (k s) -> p k s", k=4)

    with TileContext(nc) as tc:
        with (
            tc.tile_pool(name="xt", bufs=2) as pool_x,
            tc.tile_pool(name="uv", bufs=2) as pool_uv,
            tc.tile_pool(name="res", bufs=2) as pool_res,
        ):
            for t in range(NTILES):
                xt = pool_x.tile([IMGS, K], fp32)
                nc.sync.dma_start(out=xt[:], in_=x[:, t * K : (t + 1) * K])

                # vertical butterfly: row pairs (2i, 2i+1), unit-stride operands
                xv = xt[:].rearrange("p (i w) -> p i w", w=2 * W)  # (128, 16, 512)
                top = xv[:, :, 0:W]
                bot = xv[:, :, W : 2 * W]
                uv = pool_uv.tile([IMGS, K], fp32)
                u = uv[:, 0 : K // 2].rearrange("p (i w) -> p i w", w=W)
                v = uv[:, K // 2 : K].rearrange("p (i w) -> p i w", w=W)
                nc.vector.tensor_add(out=u, in0=top, in1=bot)  # a+c, b+d
                nc.vector.tensor_sub(out=v, in0=bot, in1=top)  # c-a, d-b
                # fold the Haar 1/2 on ScalarE, keeping DMAs single-dependency:
                # xt is only ever read by DVE, res only written by DVE.
                nc.scalar.mul(uv[:], uv[:], 0.5)

                # horizontal butterfly: column pairs; same op serves both halves
                uvp = uv[:].rearrange("p (n u) -> p n u", u=2)
                even = uvp[:, :, 0]
                odd = uvp[:, :, 1]
                res = pool_res.tile([IMGS, K], fp32)
                nc.vector.tensor_add(out=res[:, 0 : K // 2], in0=even, in1=odd)  # [cA|cH]
                nc.vector.tensor_sub(out=res[:, K // 2 : K], in0=odd, in1=even)  # [cV|cD]

                # res = [cA|cH|cV|cD] x 2048; one strided store to all 4 subbands
                dst = y_sub[:, :, t * OUT_K : (t + 1) * OUT_K]  # (128, 4, 2048)
                src = res[:].rearrange("p (k o) -> p k o", k=4)
                nc.sync.dma_start(out=dst, in_=src)
    # run Bacc's pass pipeline (regalloc, DCE, event-semaphore wait splitting)
    nc.compile()
    return nc


def _get_nc():
    if "nc" not in _CACHE:
        _CACHE["nc"] = build_nc()
    return _CACHE["nc"]


def kernel(x: np.ndarray) -> np.ndarray:
    from concourse.bass_utils import run_bass_kernel_spmd

    x = np.ascontiguousarray(np.asarray(x), dtype=np.float32)
    assert x.shape == (B, C, H, W), x.shape

    nc = _get_nc()
    in_maps = [
        {"x": x[c * B_PER : (c + 1) * B_PER].reshape(IMGS, IMG_PIX)}
        for c in range(N_CORES)
    ]
    results = run_bass_kernel_spmd(nc, in_maps, core_ids=list(range(N_CORES))).results
    out = np.concatenate(
        [r["y"].reshape(B_PER, C * 4, H // 2, W // 2) for r in results], axis=0
    )
    return out
